# revision 17
# baseline (speedup 1.0000x reference)
"""GAttNHP model on 8 Trainium2 NeuronCores (Bass/Tile), data-parallel over batch.

Sharding: batch 16 -> 2 batches per core x 8 cores. Group segment ops are
batch-local so no collectives are needed. Host precomputes embedding gathers,
sinusoidal time encodings and one-hot group matrices (so scatter-mean/gather
become small matmuls); the device does all the heavy matmul work in bf16 with
f32 PSUM accumulation.
"""

import os
import numpy as np
import ml_dtypes

import concourse.bass as bass
import concourse.mybir as mybir
import concourse.tile as tile
from concourse import bacc
from concourse.bass_utils import run_bass_kernel_spmd
from concourse.masks import make_identity

BF16NP = ml_dtypes.bfloat16
F32 = mybir.dt.float32
BF16 = mybir.dt.bfloat16
AF = mybir.ActivationFunctionType
ALU = mybir.AluOpType
AX = mybir.AxisListType

N_ENTITY = 8000
N_REL = 100
G = 64            # n_groups
HID = 256
D = 256           # d_model
NL = 2
NH = 4
GPD = 64          # group proj dim
GH = 2
D_TOTAL = D * NL          # 512
D_FEAT = D_TOTAL + 2 * HID  # 1024
D_MG = D_FEAT + GPD       # 1088
B, L = 16, 512
LH = L - 1                # 511
NCORES = 8
BL = B // NCORES          # 2 batches per core
ECH = 500                 # entity chunk size
NCH = N_ENTITY // ECH     # 16

# event tiles of 511
EVT = [(t * 128, min(128, LH - t * 128)) for t in range(4)]  # (start, size)

LAST_RESULTS = None  # test.py reads exec_time_ns from here
LAST_RUN_WALL_NS = None


# ---------------------------------------------------------------- host prep

def _time_enc(t, d=D):
    i = np.arange(d // 2)
    freqs = np.exp(-np.log(10000.0) * (2.0 * i / d)).astype(np.float32)
    ang = t[..., None].astype(np.float32) * freqs
    return np.concatenate([np.sin(ang), np.cos(ang)], axis=-1).astype(np.float32)


def _bf16(a):
    return np.ascontiguousarray(np.asarray(a, np.float32).astype(BF16NP))


def _f32(a):
    return np.ascontiguousarray(np.asarray(a, np.float32))


# ------------------------------------------------------------- device build

def _emit(nc, tc, t):
    """Emit the whole per-core program. `t` is the dict of DRAM handles."""
    from contextlib import ExitStack
    ctx = ExitStack()
    const = ctx.enter_context(tc.tile_pool(name="const", bufs=1))
    acts = ctx.enter_context(tc.tile_pool(name="acts", bufs=1))
    work = ctx.enter_context(tc.tile_pool(name="work", bufs=2))
    grp = ctx.enter_context(tc.tile_pool(name="grp", bufs=1))
    stream = ctx.enter_context(tc.tile_pool(name="stream", bufs=3))
    outp = ctx.enter_context(tc.tile_pool(name="outp", bufs=4))
    psum = ctx.enter_context(tc.tile_pool(name="psum", bufs=2, space="PSUM"))

    # ---- constants ----
    ident16 = const.tile([128, 128], BF16, name="ident16")
    make_identity(nc, ident16)
    ident32 = const.tile([128, 128], F32, name="ident32")
    make_identity(nc, ident32)
    ones_row = const.tile([1, 512], BF16, name="ones_row")
    nc.gpsimd.memset(ones_row[:], 1.0)
    ones_ev = const.tile([128, 512], BF16, name="ones_ev")
    nc.gpsimd.memset(ones_ev[:], 1.0)
    eps_c = const.tile([GPD, 1], F32, name="eps_c")
    nc.gpsimd.memset(eps_c[:], 1e-5)

    # attention core weights: [l][ki tiles of 128, 256]
    wqkvo = {}
    for wname in ("wq", "wk", "wv", "wo"):
        for l in range(NL):
            w_sb = const.tile([128, 2, 256], BF16, name=f"{wname}{l}")
            nc.sync.dma_start(w_sb[:], t[wname][l].rearrange("(ki p) n -> p ki n", p=128))
            wqkvo[(wname, l)] = w_sb

    gpw_sb = const.tile([128, 8, GPD], BF16, name="gpw")
    nc.sync.dma_start(gpw_sb[:], t["gpw"][:].rearrange("(kt p) n -> p kt n", p=128))
    gpbr_sb = const.tile([1, GPD], BF16, name="gpbr")
    nc.sync.dma_start(gpbr_sb[:], t["gpbr"][:])
    gpbc_sb = const.tile([GPD, 1], F32, name="gpbc")
    nc.sync.dma_start(gpbc_sb[:], t["gpbc"][:])

    small_w = {}
    for wname, shp in (("gainw", [GPD, 3 * GPD]), ("gaoutw", [GPD, GPD]),
                       ("ffw1", [GPD, GPD]), ("ffw2", [GPD, GPD])):
        w_sb = const.tile(shp, BF16, name=wname)
        nc.sync.dma_start(w_sb[:], t[wname][:])
        small_w[wname] = w_sb
    for bname, n in (("gainbr", 3 * GPD), ("gaoutbr", GPD), ("ffb1r", GPD), ("ffb2r", GPD)):
        b_sb = const.tile([1, n], BF16, name=bname)
        nc.sync.dma_start(b_sb[:], t[bname][:])
        small_w[bname] = b_sb
    for lname in ("n1wt", "n1bt", "n2wt", "n2bt"):
        l_sb = const.tile([GPD, GPD], F32, name=lname)
        nc.sync.dma_start(l_sb[:], t[lname][:])
        small_w[lname] = l_sb

    mgw_sb = const.tile([128, 9, D_FEAT], BF16, name="mgw")
    nc.sync.dma_start(mgw_sb[:, :8, :],
                      t["mgw"][:D_FEAT].rearrange("(kt p) n -> p kt n", p=128))
    nc.sync.dma_start(mgw_sb[:64, 8, :], t["mgw"][D_FEAT:D_MG])
    mgbc_sb = const.tile([128, 8, 1], F32, name="mgbc")
    nc.sync.dma_start(mgbc_sb[:], t["mgbc"][:].rearrange("(f p) o -> p f o", p=128))


    # per-(b) long-lived activation tiles
    enhT = {}     # [128, 8, 511] bf16 per b
    merged_src = {}   # per b: list of (ap, ksz) for the 9 contraction tiles of mg

    for b in range(BL):
        xT = acts.tile([128, 2, LH], BF16, name=f"xT{b}")
        nc.sync.dma_start(xT[:], t["xt"][b].rearrange("(ki p) e -> p ki e", p=128))
        curT = acts.tile([128, 2, LH], BF16, name=f"c0T{b}")
        nc.sync.dma_start(curT[:], t["c0t"][b].rearrange("(ki p) e -> p ki e", p=128))

        layer_T = []
        for l in range(NL):
            # ---- projections ----
            qT = work.tile([128, 2, LH], BF16, name="qT", tag="qT")
            kT = work.tile([128, 2, LH], BF16, name="kT", tag="kT")
            for dst, wname in ((qT, "wq"), (kT, "wk")):
                src = curT if wname == "wq" else xT
                w_sb = wqkvo[(wname, l)]
                for ko in range(2):
                    ps = psum.tile([128, 512], F32, name="ps_proj", tag="ps_big")
                    for ki in range(2):
                        nc.tensor.matmul(ps[:, :LH], w_sb[:, ki, ko * 128:(ko + 1) * 128],
                                         src[:, ki, :], start=(ki == 0), stop=(ki == 1))
                    nc.scalar.copy(dst[:, ko, :], ps[:, :LH])
            # v row-major, padded per head with a ones column: [128, kt, h, 65]
            v_sb = work.tile([128, 4, NH, 65], BF16, name="v_sb", tag="v_sb")
            nc.gpsimd.memset(v_sb[:, :, :, 64:65], 1.0)
            w_sb = wqkvo[("wv", l)]
            for et, (e0, esz) in enumerate(EVT):
                ps = psum.tile([128, 512], F32, name="ps_v", tag="ps_big")
                for ki in range(2):
                    nc.tensor.matmul(ps[:esz, :256], xT[:, ki, e0:e0 + esz],
                                     w_sb[:, ki, :], start=(ki == 0), stop=(ki == 1))
                nc.scalar.copy(v_sb[:esz, et, :, 0:64],
                               ps[:esz, :256].rearrange("p (h d) -> p h d", h=NH))

            # ---- scores E^T = exp(S^T/8) with causal zeroing ----
            E = work.tile([128, NH, 4, LH], BF16, name="E", tag="E", bufs=1)
            for h in range(NH):
                hp = (64 * h) % 128
                ht = h // 2
                for kt, (k0, ksz) in enumerate(EVT):
                    qlo = k0
                    qn = LH - qlo
                    ps = psum.tile([128, 512], F32, name="ps_s", tag="ps_big")
                    nc.tensor.matmul(ps[:ksz, :qn], kT[hp:hp + 64, ht, k0:k0 + ksz],
                                     qT[hp:hp + 64, ht, qlo:LH], start=True, stop=True)
                    nc.scalar.activation(E[:ksz, h, kt, qlo:LH], ps[:ksz, :qn],
                                         AF.Exp, scale=0.125)
                    # zero strictly-upper part of the diagonal block (q < k)
                    w = min(qn, 128)
                    nc.gpsimd.affine_select(
                        out=E[:ksz, h, kt, qlo:qlo + w], in_=E[:ksz, h, kt, qlo:qlo + w],
                        compare_op=ALU.is_ge, fill=0.0, base=0,
                        pattern=[[1, w]], channel_multiplier=-1)

            # ---- O = softmax @ V (row-major) ----
            attn = work.tile([128, 4, 256], BF16, name="attn", tag="attn")
            for qt, (q0, qsz) in enumerate(EVT):
                ps_o = psum.tile([128, NH, 65], F32, name="ps_o", tag="ps_o")
                for h in range(NH):
                    for kt in range(qt + 1):
                        k0, ksz = EVT[kt]
                        nc.tensor.matmul(ps_o[:qsz, h, :], E[:ksz, h, kt, q0:q0 + qsz],
                                         v_sb[:ksz, kt, h, :],
                                         start=(kt == 0), stop=(kt == qt))
                rs = work.tile([128, NH, 1], F32, name="rs", tag="rs")
                nc.vector.reciprocal(rs[:qsz], ps_o[:qsz, :, 64:65])
                for h in range(NH):
                    nc.vector.tensor_scalar_mul(attn[:qsz, qt, 64 * h:64 * (h + 1)],
                                                ps_o[:qsz, h, 0:64], rs[:qsz, h])

            # ---- attn^T via PE transpose ----
            attnT = work.tile([128, 2, LH], BF16, name="attnT", tag="attnT")
            for ft in range(2):
                for qt, (q0, qsz) in enumerate(EVT):
                    ps_t = psum.tile([128, 128], BF16, name="ps_t", tag="ps_t")
                    nc.tensor.transpose(ps_t[:128, :qsz],
                                        attn[:qsz, qt, ft * 128:(ft + 1) * 128],
                                        ident16[:qsz, :qsz])
                    nc.scalar.copy(attnT[:, ft, q0:q0 + qsz], ps_t[:128, :qsz])

            # ---- cur = cur + attn @ Wo (feature-major) ----
            w_sb = wqkvo[("wo", l)]
            ncurT = acts.tile([128, 2, LH], BF16, name=f"c{l + 1}T{b}")
            for ko in range(2):
                ps = psum.tile([128, 512], F32, name="ps_wo", tag="ps_big")
                for ki in range(2):
                    nc.tensor.matmul(ps[:, :LH], w_sb[:, ki, ko * 128:(ko + 1) * 128],
                                     attnT[:, ki, :], start=(ki == 0), stop=(ki == 1))
                nc.vector.tensor_add(ncurT[:, ko, :], ps[:, :LH], curT[:, ko, :])
            curT = ncurT
            layer_T.append(ncurT)

        c1T, c2T = layer_T

        # ---- enc row-major (for the scatter matmul) ----
        enc_row = acts.tile([128, 4, D_TOTAL], BF16, name=f"enc_row{b}")
        for l, cT in enumerate(layer_T):
            for ft in range(2):
                for qt, (q0, qsz) in enumerate(EVT):
                    ps_t = psum.tile([128, 128], BF16, name="ps_t2", tag="ps_t")
                    nc.tensor.transpose(ps_t[:qsz, :128], cT[:, ft, q0:q0 + qsz],
                                        ident16[:, :128])
                    nc.scalar.copy(enc_row[:qsz, qt, l * 256 + ft * 128:l * 256 + (ft + 1) * 128],
                                   ps_t[:qsz, :128])

        # ---- scatter-mean -> rep^T [1024, 64] (enc block on device, s/r block host) ----
        asc_sb = grp.tile([128, 4, G], BF16, name="asc", tag=f"asc{b}")
        nc.sync.dma_start(asc_sb[:], t["asc"][b].rearrange("(et p) g -> p et g", p=128))
        repT = acts.tile([128, 8, G], BF16, name=f"repT{b}")
        nc.sync.dma_start(repT[:, 4:8, :],
                          t["repsrt"][b].rearrange("(ft p) g -> p ft g", p=128))
        for ft in range(4):
            ps = psum.tile([128, G], F32, name="ps_rep", tag="ps_o")
            for et, (e0, esz) in enumerate(EVT):
                nc.tensor.matmul(ps[:, :], enc_row[:esz, et, ft * 128:(ft + 1) * 128],
                                 asc_sb[:esz, et, :], start=(et == 0), stop=(et == 3))
            nc.scalar.copy(repT[:, ft, :], ps[:, :])

        # ---- gp (both layouts) ----
        gpT = grp.tile([GPD, GPD], BF16, name="gpT", tag=f"gpT{b}")
        ps1 = psum.tile([GPD, GPD], F32, name="ps_gpT", tag="ps_o")
        for kt in range(8):
            nc.tensor.matmul(ps1[:, :], gpw_sb[:, kt, :], repT[:, kt, :],
                             start=(kt == 0), stop=(kt == 7))
        nc.scalar.activation(gpT[:], ps1[:], AF.Identity, bias=gpbc_sb[:])
        gp_r = grp.tile([GPD, GPD], F32, name="gp_r", tag=f"gp_r{b}")
        ps2 = psum.tile([GPD, GPD], F32, name="ps_gp", tag="ps_o")
        for kt in range(8):
            nc.tensor.matmul(ps2[:, :], repT[:, kt, :], gpw_sb[:, kt, :],
                             start=(kt == 0), stop=False)
        nc.tensor.matmul(ps2[:, :], ones_row[:, :GPD], gpbr_sb[:, :],
                         start=False, stop=True)
        nc.vector.tensor_copy(gp_r[:], ps2[:])

        # ---- group attention ----
        qkv = grp.tile([GPD, 3 * GPD], BF16, name="qkv", tag=f"qkv{b}")
        ps = psum.tile([GPD, 3 * GPD], F32, name="ps_qkv", tag="ps_big")
        nc.tensor.matmul(ps[:, :], gpT[:], small_w["gainw"][:], start=True, stop=False)
        nc.tensor.matmul(ps[:, :], ones_row[:, :GPD], small_w["gainbr"][:],
                         start=False, stop=True)
        nc.scalar.copy(qkv[:], ps[:, :])
        # per-head transposed q/k: j=0,1 -> q heads, j=2,3 -> k heads
        qkT = grp.tile([32, 4, GPD], BF16, name="qkT", tag=f"qkT{b}")
        for j in range(4):
            ps_t = psum.tile([32, 128], BF16, name="ps_qt", tag="ps_t")
            nc.tensor.transpose(ps_t[:32, :GPD], qkv[:, 32 * j:32 * (j + 1)],
                                ident16[:GPD, :GPD])
            nc.scalar.copy(qkT[:, j, :], ps_t[:32, :GPD])

        gaT = grp.tile([GPD, GPD], BF16, name="gaT", tag=f"gaT{b}")
        for h in range(GH):
            ps_s = psum.tile([GPD, GPD], F32, name="ps_sg", tag="ps_o")
            nc.tensor.matmul(ps_s[:, :], qkT[:, h, :], qkT[:, 2 + h, :],
                             start=True, stop=True)
            P_sb = grp.tile([GPD, GPD], F32, name="P_sb", tag="P_sb")
            sum_c = grp.tile([GPD, 1], F32, name="sum_c", tag="sum_c")
            nc.scalar.activation(P_sb[:], ps_s[:, :], AF.Exp,
                                 scale=float(1.0 / np.sqrt(32.0)),
                                 accum_out=sum_c[:])
            rsum = grp.tile([GPD, 1], F32, name="rsum", tag="rsum")
            nc.vector.reciprocal(rsum[:], sum_c[:])
            Pn = grp.tile([GPD, GPD], BF16, name="Pn", tag="Pn")
            nc.vector.tensor_scalar_mul(Pn[:], P_sb[:], rsum[:])
            ps_pt = psum.tile([GPD, GPD], BF16, name="ps_pt", tag="ps_t")
            nc.tensor.transpose(ps_pt[:, :], Pn[:], ident16[:GPD, :GPD])
            PnT = grp.tile([GPD, GPD], BF16, name="PnT", tag="PnT")
            nc.scalar.copy(PnT[:], ps_pt[:, :])
            ps_og = psum.tile([32, GPD], F32, name="ps_og", tag="ps_t")
            nc.tensor.matmul(ps_og[:, :], qkv[:, 128 + 32 * h:128 + 32 * (h + 1)],
                             PnT[:], start=True, stop=True)
            nc.scalar.copy(gaT[32 * h:32 * (h + 1), :], ps_og[:, :])

        ps_ga = psum.tile([GPD, GPD], F32, name="ps_ga", tag="ps_o")
        nc.tensor.matmul(ps_ga[:, :], gaT[:], small_w["gaoutw"][:], start=True, stop=False)
        nc.tensor.matmul(ps_ga[:, :], ones_row[:, :GPD], small_w["gaoutbr"][:],
                         start=False, stop=True)
        y1 = grp.tile([GPD, GPD], F32, name="y1", tag=f"y1{b}")
        nc.vector.tensor_add(y1[:], ps_ga[:, :], gp_r[:])

        def layer_norm(y, wt, bt, out_dtype, out_name):
            mu = grp.tile([GPD, 1], F32, name=f"{out_name}_mu", tag=f"{out_name}_mu")
            nc.vector.reduce_sum(mu[:], y[:], axis=AX.X)
            nc.scalar.mul(mu[:], mu[:], 1.0 / GPD)
            xc = grp.tile([GPD, GPD], F32, name=f"{out_name}_xc", tag=f"{out_name}_xc")
            nc.vector.tensor_scalar_sub(xc[:], y[:], mu[:])
            sq = grp.tile([GPD, GPD], F32, name=f"{out_name}_sq", tag=f"{out_name}_sq")
            vs = grp.tile([GPD, 1], F32, name=f"{out_name}_vs", tag=f"{out_name}_vs")
            nc.scalar.activation(sq[:], xc[:], AF.Square, accum_out=vs[:])
            # rstd = exp(-0.5 * ln(var + eps)); avoids the Sqrt act table
            lv = grp.tile([GPD, 1], F32, name=f"{out_name}_lv", tag=f"{out_name}_lv")
            nc.scalar.activation(lv[:], vs[:], AF.Ln, scale=1.0 / GPD, bias=eps_c[:])
            rstd = grp.tile([GPD, 1], F32, name=f"{out_name}_rstd", tag=f"{out_name}_rstd")
            nc.scalar.activation(rstd[:], lv[:], AF.Exp, scale=-0.5)
            xh = grp.tile([GPD, GPD], F32, name=f"{out_name}_xh", tag=f"{out_name}_xh")
            nc.vector.scalar_tensor_tensor(xh[:], xc[:], rstd[:], wt[:],
                                           op0=ALU.mult, op1=ALU.mult)
            out = grp.tile([GPD, GPD], out_dtype, name=out_name, tag=f"{out_name}{b}")
            nc.vector.tensor_add(out[:], xh[:], bt[:])
            return out

        gn = layer_norm(y1, small_w["n1wt"], small_w["n1bt"], F32, "gn")
        ps_gt = psum.tile([GPD, GPD], F32, name="ps_gnT", tag="ps_t")
        nc.tensor.transpose(ps_gt[:, :], gn[:], ident32[:GPD, :GPD])
        gnT = grp.tile([GPD, GPD], BF16, name="gnT", tag=f"gnT{b}")
        nc.scalar.copy(gnT[:], ps_gt[:, :])

        ps_f1 = psum.tile([GPD, GPD], F32, name="ps_f1", tag="ps_o")
        nc.tensor.matmul(ps_f1[:, :], gnT[:], small_w["ffw1"][:], start=True, stop=False)
        nc.tensor.matmul(ps_f1[:, :], ones_row[:, :GPD], small_w["ffb1r"][:],
                         start=False, stop=True)
        ff1 = grp.tile([GPD, GPD], BF16, name="ff1", tag="ff1")
        nc.scalar.activation(ff1[:], ps_f1[:, :], AF.Relu)
        ps_ft = psum.tile([GPD, GPD], BF16, name="ps_ff1T", tag="ps_t")
        nc.tensor.transpose(ps_ft[:, :], ff1[:], ident16[:GPD, :GPD])
        ff1T = grp.tile([GPD, GPD], BF16, name="ff1T", tag="ff1T")
        nc.scalar.copy(ff1T[:], ps_ft[:, :])
        ps_f2 = psum.tile([GPD, GPD], F32, name="ps_f2", tag="ps_o")
        nc.tensor.matmul(ps_f2[:, :], ff1T[:], small_w["ffw2"][:], start=True, stop=False)
        nc.tensor.matmul(ps_f2[:, :], ones_row[:, :GPD], small_w["ffb2r"][:],
                         start=False, stop=True)
        y2 = grp.tile([GPD, GPD], F32, name="y2", tag=f"y2{b}")
        nc.vector.tensor_add(y2[:], ps_f2[:, :], gn[:])

        gout = layer_norm(y2, small_w["n2wt"], small_w["n2bt"], BF16, "gout")

        # ---- gather back to events: gathered^T [64, 511] ----
        a01t_sb = grp.tile([G, LH], BF16, name="a01t", tag=f"a01t{b}")
        nc.sync.dma_start(a01t_sb[:], t["a01t"][b])
        gatheredT = acts.tile([G, LH], BF16, name=f"gatheredT{b}")
        ps_gth = psum.tile([GPD, 512], F32, name="ps_gth", tag="ps_big")
        nc.tensor.matmul(ps_gth[:, :LH], gout[:], a01t_sb[:], start=True, stop=True)
        nc.scalar.copy(gatheredT[:], ps_gth[:, :LH])

        # ---- s/r embedding broadcast rows of merged^T ----
        srt_sb = grp.tile([128, 4, 1], F32, name="srt", tag=f"srt{b}")
        nc.sync.dma_start(srt_sb[:], t["srt"][b].rearrange("(j p) o -> p j o", p=128))
        srb = acts.tile([128, 4, LH], BF16, name=f"srb{b}")
        for j in range(4):
            nc.vector.tensor_scalar_mul(srb[:, j, :], ones_ev[:, :LH], srt_sb[:, j])

        merged_src[b] = [
            (c1T[:, 0, :], 128), (c1T[:, 1, :], 128),
            (c2T[:, 0, :], 128), (c2T[:, 1, :], 128),
            (srb[:, 0, :], 128), (srb[:, 1, :], 128),
            (srb[:, 2, :], 128), (srb[:, 3, :], 128),
            (gatheredT[:, :], 64),
        ]

        # ---- merge matmul -> enhanced^T [1024, 511] ----
        eT = acts.tile([128, 8, LH], BF16, name=f"enhT{b}")
        for fo in range(8):
            ps = psum.tile([128, 512], F32, name="ps_mg", tag="ps_big")
            for kt in range(9):
                src, ksz = merged_src[b][kt]
                nc.tensor.matmul(ps[:, :LH], mgw_sb[:ksz, kt, fo * 128:(fo + 1) * 128],
                                 src, start=(kt == 0), stop=(kt == 8))
            nc.scalar.activation(eT[:, fo, :], ps[:, :LH], AF.Identity,
                                 bias=mgbc_sb[:, fo])
        enhT[b] = eT

    # ---- intensity head, streaming int_w ----
    for c in range(NCH):
        intw_t = stream.tile([128, 8, ECH], BF16, name="intw_t", tag="intw")
        nc.sync.dma_start(intw_t[:],
                          t["intw"][:, c * ECH:(c + 1) * ECH]
                          .rearrange("(kt p) e -> p kt e", p=128))
        intb_t = stream.tile([1, ECH], BF16, name="intb_t", tag="intb", bufs=2)
        nc.sync.dma_start(intb_t[:], t["intbr"][:, c * ECH:(c + 1) * ECH])
        for b in range(BL):
            for qt, (q0, qsz) in enumerate(EVT):
                ps = psum.tile([128, ECH], F32, name="ps_int", tag="ps_int")
                for kt in range(8):
                    nc.tensor.matmul(ps[:qsz, :], enhT[b][:, kt, q0:q0 + qsz],
                                     intw_t[:, kt, :], start=(kt == 0), stop=False)
                nc.tensor.matmul(ps[:qsz, :], ones_row[:, :qsz], intb_t[:],
                                 start=False, stop=True)
                # softplus(x) = ln(exp(x) + 1); logits are O(1) so no overflow
                e_t = outp.tile([128, ECH], F32, name="e_t", tag="e_t", bufs=2)
                nc.scalar.activation(e_t[:qsz], ps[:qsz, :], AF.Exp)
                o_t = outp.tile([128, ECH], F32, name="o_t", tag="o_t")
                nc.scalar.activation(o_t[:qsz], e_t[:qsz], AF.Ln, bias=1.0)
                nc.sync.dma_start(t["out"][b, q0:q0 + qsz, c * ECH:(c + 1) * ECH],
                                  o_t[:qsz])
    ctx.close()


def _build_program():
    nc = bacc.Bacc("TRN2", target_bir_lowering=False, debug=False)
    d = {}

    def inp(name, shape, dt):
        d[name] = nc.dram_tensor(name, list(shape), dt, kind="ExternalInput")

    inp("xt", (BL, D, LH), BF16)
    inp("c0t", (BL, D, LH), BF16)
    for w in ("wq", "wk", "wv", "wo"):
        inp(w, (NL, D, D), BF16)
    inp("gpw", (D_FEAT, GPD), BF16)
    inp("gpbr", (1, GPD), BF16)
    inp("gpbc", (GPD, 1), F32)
    inp("gainw", (GPD, 3 * GPD), BF16)
    inp("gainbr", (1, 3 * GPD), BF16)
    inp("gaoutw", (GPD, GPD), BF16)
    inp("gaoutbr", (1, GPD), BF16)
    inp("ffw1", (GPD, GPD), BF16)
    inp("ffb1r", (1, GPD), BF16)
    inp("ffw2", (GPD, GPD), BF16)
    inp("ffb2r", (1, GPD), BF16)
    for n in ("n1wt", "n1bt", "n2wt", "n2bt"):
        inp(n, (GPD, GPD), F32)
    inp("mgw", (D_MG, D_FEAT), BF16)
    inp("mgbc", (D_FEAT, 1), F32)
    inp("intw", (D_FEAT, N_ENTITY), BF16)
    inp("intbr", (1, N_ENTITY), BF16)
    inp("asc", (BL, 512, G), BF16)
    inp("a01t", (BL, G, LH), BF16)
    inp("srt", (BL, 512, 1), F32)
    inp("repsrt", (BL, 512, G), BF16)
    d["out"] = nc.dram_tensor("out", [BL, LH, N_ENTITY], F32, kind="ExternalOutput")

    with tile.TileContext(nc) as tc:
        _emit(nc, tc, d)
    nc.compile()
    return nc


_PROG = None


def _get_program():
    global _PROG
    if _PROG is None:
        _PROG = _build_program()
    return _PROG


# -------------------------------------------------------------------- kernel

def prepare_in_maps(subs, marks, objs, times, dt, mask, group_map,
                    obj_embed, core_Wq, core_Wk, core_Wv, core_Wo,
                    sub_embed, rel_embed, gp_w, gp_b, ga_in_w, ga_in_b,
                    ga_out_w, ga_out_b, ffn_w1, ffn_b1, ffn_w2, ffn_b2,
                    n1_w, n1_b, n2_w, n2_b, mg_w, mg_b, int_w, int_b):
    subs = np.asarray(subs).astype(np.int64)
    marks = np.asarray(marks).astype(np.int64)
    objs = np.asarray(objs).astype(np.int64)
    times = _f32(times)
    dt = _f32(dt)
    mask = np.asarray(mask).astype(bool)
    group_map = np.asarray(group_map).astype(np.int64)
    obj_embed = _f32(obj_embed)
    sub_embed = _f32(sub_embed)
    rel_embed = _f32(rel_embed)

    hist = objs[:, :-1]
    x = obj_embed[hist] + _time_enc(times[:, :-1]) + _time_enc(dt[:, :-1])
    cur0 = _time_enc(times[:, 1:])
    xT = _bf16(x.transpose(0, 2, 1))          # [B, 256, 511]
    c0T = _bf16(cur0.transpose(0, 2, 1))

    g_ids = group_map[subs[:, :-1] * N_REL + marks[:, :-1]]   # [B, 511]
    fm = mask[:, :-1].astype(np.float32)
    a01 = np.zeros((B, LH, G), np.float32)
    bi = np.arange(B)[:, None].repeat(LH, 1)
    ei = np.arange(LH)[None, :].repeat(B, 0)
    a01[bi.ravel(), ei.ravel(), g_ids.ravel()] = fm.ravel()
    cnt = a01.sum(axis=1)                                     # [B, G]
    asc = a01 / np.maximum(cnt, 1.0)[:, None, :]
    asc_pad = np.zeros((B, 512, G), np.float32)
    asc_pad[:, :LH] = asc
    a01t = a01.transpose(0, 2, 1)                             # [B, G, 511]

    s_emb = sub_embed[subs[:, 0]]                             # [B, 256]
    r_emb = rel_embed[marks[:, 0]]                            # [B, 256]
    sr = np.concatenate([s_emb, r_emb], axis=1)               # [B, 512]
    ind = (cnt > 0).astype(np.float32)                        # [B, G]
    repsrt = sr[:, :, None] * ind[:, None, :]                 # [B, 512, G]

    shared = {
        "wq": _bf16(core_Wq), "wk": _bf16(core_Wk),
        "wv": _bf16(core_Wv), "wo": _bf16(core_Wo),
        "gpw": _bf16(gp_w), "gpbr": _bf16(np.asarray(gp_b).reshape(1, GPD)),
        "gpbc": _f32(np.asarray(gp_b).reshape(GPD, 1)),
        "gainw": _bf16(ga_in_w), "gainbr": _bf16(np.asarray(ga_in_b).reshape(1, -1)),
        "gaoutw": _bf16(ga_out_w), "gaoutbr": _bf16(np.asarray(ga_out_b).reshape(1, -1)),
        "ffw1": _bf16(ffn_w1), "ffb1r": _bf16(np.asarray(ffn_b1).reshape(1, -1)),
        "ffw2": _bf16(ffn_w2), "ffb2r": _bf16(np.asarray(ffn_b2).reshape(1, -1)),
        "n1wt": _f32(np.broadcast_to(np.asarray(n1_w, np.float32), (GPD, GPD))),
        "n1bt": _f32(np.broadcast_to(np.asarray(n1_b, np.float32), (GPD, GPD))),
        "n2wt": _f32(np.broadcast_to(np.asarray(n2_w, np.float32), (GPD, GPD))),
        "n2bt": _f32(np.broadcast_to(np.asarray(n2_b, np.float32), (GPD, GPD))),
        "mgw": _bf16(mg_w), "mgbc": _f32(np.asarray(mg_b).reshape(D_FEAT, 1)),
        "intw": _bf16(int_w), "intbr": _bf16(np.asarray(int_b).reshape(1, N_ENTITY)),
    }

    in_maps = []
    for core in range(NCORES):
        bs = slice(core * BL, (core + 1) * BL)
        m = dict(shared)
        m["xt"] = xT[bs]
        m["c0t"] = c0T[bs]
        m["asc"] = _bf16(asc_pad[bs])
        m["a01t"] = _bf16(a01t[bs])
        m["srt"] = _f32(sr[bs][:, :, None])
        m["repsrt"] = _bf16(repsrt[bs])
        in_maps.append(m)
    return in_maps


def kernel(**inputs):
    global LAST_RESULTS, LAST_RUN_WALL_NS
    import time
    in_maps = prepare_in_maps(**inputs)
    nc = _get_program()
    t0 = time.perf_counter()
    res = run_bass_kernel_spmd(nc, in_maps, core_ids=list(range(NCORES)))
    LAST_RUN_WALL_NS = int((time.perf_counter() - t0) * 1e9)
    LAST_RESULTS = res
    out = np.concatenate([res.results[i]["out"] for i in range(NCORES)], axis=0)
    return np.ascontiguousarray(out.reshape(B, LH, N_ENTITY).astype(np.float32))


# revision 22
# speedup vs baseline: 1.4440x; 1.4440x over previous
"""GAttNHP model on 8 Trainium2 NeuronCores (Bass/Tile), data-parallel over batch.

Sharding: batch 16 -> 2 batches per core x 8 cores. Group segment ops are
batch-local so no collectives are needed. Host precomputes embedding gathers,
sinusoidal time encodings and one-hot group matrices (so scatter-mean/gather
become small matmuls); the device does all the heavy matmul work in bf16 with
f32 PSUM accumulation.
"""

import os
import numpy as np
import ml_dtypes

import concourse.bass as bass
import concourse.mybir as mybir
import concourse.tile as tile
from concourse import bacc
from concourse.bass_utils import run_bass_kernel_spmd
from concourse.masks import make_identity

BF16NP = ml_dtypes.bfloat16
F32 = mybir.dt.float32
BF16 = mybir.dt.bfloat16
AF = mybir.ActivationFunctionType
ALU = mybir.AluOpType
AX = mybir.AxisListType

N_ENTITY = 8000
N_REL = 100
G = 64            # n_groups
HID = 256
D = 256           # d_model
NL = 2
NH = 4
GPD = 64          # group proj dim
GH = 2
D_TOTAL = D * NL          # 512
D_FEAT = D_TOTAL + 2 * HID  # 1024
D_MG = D_FEAT + GPD       # 1088
B, L = 16, 512
LH = L - 1                # 511
NCORES = 8
BL = B // NCORES          # 2 batches per core
ECH = 1000                # entity chunk size (2 psum banks wide)
NCH = N_ENTITY // ECH     # 8

# event tiles of 511
EVT = [(t * 128, min(128, LH - t * 128)) for t in range(4)]  # (start, size)

LAST_RESULTS = None  # test.py reads exec_time_ns from here
LAST_RUN_WALL_NS = None


# ---------------------------------------------------------------- host prep

def _time_enc(t, d=D):
    i = np.arange(d // 2)
    freqs = np.exp(-np.log(10000.0) * (2.0 * i / d)).astype(np.float32)
    ang = t[..., None].astype(np.float32) * freqs
    return np.concatenate([np.sin(ang), np.cos(ang)], axis=-1).astype(np.float32)


def _bf16(a):
    return np.ascontiguousarray(np.asarray(a, np.float32).astype(BF16NP))


def _f32(a):
    return np.ascontiguousarray(np.asarray(a, np.float32))


# ------------------------------------------------------------- device build

def _emit(nc, tc, t):
    """Emit the whole per-core program. `t` is the dict of DRAM handles."""
    from contextlib import ExitStack
    ctx = ExitStack()
    const = ctx.enter_context(tc.tile_pool(name="const", bufs=1))
    acts = ctx.enter_context(tc.tile_pool(name="acts", bufs=1))
    work = ctx.enter_context(tc.tile_pool(name="work", bufs=2))
    grp = ctx.enter_context(tc.tile_pool(name="grp", bufs=1))
    stream = ctx.enter_context(tc.tile_pool(name="stream", bufs=3))
    outp = ctx.enter_context(tc.tile_pool(name="outp", bufs=4))
    psum = ctx.enter_context(tc.tile_pool(name="psum", bufs=2, space="PSUM"))

    # ---- constants ----
    ident16 = const.tile([128, 128], BF16, name="ident16")
    make_identity(nc, ident16)
    ident32 = const.tile([128, 128], F32, name="ident32")
    make_identity(nc, ident32)
    ones_row = const.tile([1, 512], BF16, name="ones_row")
    nc.gpsimd.memset(ones_row[:], 1.0)
    ones_ev = const.tile([128, 512], BF16, name="ones_ev")
    nc.gpsimd.memset(ones_ev[:], 1.0)
    eps_c = const.tile([GPD, 1], F32, name="eps_c")
    nc.gpsimd.memset(eps_c[:], 1e-5)

    # attention core weights: [l][ki tiles of 128, 256]
    wqkvo = {}
    for wname in ("wq", "wk", "wv", "wo"):
        for l in range(NL):
            w_sb = const.tile([128, 2, 256], BF16, name=f"{wname}{l}")
            nc.sync.dma_start(w_sb[:], t[wname][l].rearrange("(ki p) n -> p ki n", p=128))
            wqkvo[(wname, l)] = w_sb

    gpw_sb = const.tile([128, 8, GPD], BF16, name="gpw")
    nc.sync.dma_start(gpw_sb[:], t["gpw"][:].rearrange("(kt p) n -> p kt n", p=128))
    gpbr_sb = const.tile([1, GPD], BF16, name="gpbr")
    nc.sync.dma_start(gpbr_sb[:], t["gpbr"][:])
    gpbc_sb = const.tile([GPD, 1], F32, name="gpbc")
    nc.sync.dma_start(gpbc_sb[:], t["gpbc"][:])

    small_w = {}
    for wname, shp in (("gainw", [GPD, 3 * GPD]), ("gaoutw", [GPD, GPD]),
                       ("ffw1", [GPD, GPD]), ("ffw2", [GPD, GPD])):
        w_sb = const.tile(shp, BF16, name=wname)
        nc.sync.dma_start(w_sb[:], t[wname][:])
        small_w[wname] = w_sb
    for bname, n in (("gainbr", 3 * GPD), ("gaoutbr", GPD), ("ffb1r", GPD), ("ffb2r", GPD)):
        b_sb = const.tile([1, n], BF16, name=bname)
        nc.sync.dma_start(b_sb[:], t[bname][:])
        small_w[bname] = b_sb
    for lname in ("n1wt", "n1bt", "n2wt", "n2bt"):
        l_sb = const.tile([GPD, GPD], F32, name=lname)
        nc.sync.dma_start(l_sb[:], t[lname][:])
        small_w[lname] = l_sb

    mgw_sb = const.tile([128, 9, D_FEAT], BF16, name="mgw")
    nc.sync.dma_start(mgw_sb[:, :8, :],
                      t["mgw"][:D_FEAT].rearrange("(kt p) n -> p kt n", p=128))
    nc.sync.dma_start(mgw_sb[:64, 8, :], t["mgw"][D_FEAT:D_MG])
    mgbc_sb = const.tile([128, 8, 1], F32, name="mgbc")
    nc.sync.dma_start(mgbc_sb[:], t["mgbc"][:].rearrange("(f p) o -> p f o", p=128))


    # per-(b) long-lived activation tiles
    enhT = {}     # [128, 8, 511] bf16 per b
    merged_src = {}   # per b: list of (ap, ksz) for the 9 contraction tiles of mg

    for b in range(BL):
        xT = acts.tile([128, 2, LH], BF16, name=f"xT{b}")
        nc.sync.dma_start(xT[:], t["xt"][b].rearrange("(ki p) e -> p ki e", p=128))
        curT = acts.tile([128, 2, LH], BF16, name=f"c0T{b}")
        nc.sync.dma_start(curT[:], t["c0t"][b].rearrange("(ki p) e -> p ki e", p=128))

        layer_T = []
        for l in range(NL):
            # ---- projections ----
            qT = work.tile([128, 2, LH], BF16, name="qT", tag="qT")
            kT = work.tile([128, 2, LH], BF16, name="kT", tag="kT")
            for dst, wname in ((qT, "wq"), (kT, "wk")):
                src = curT if wname == "wq" else xT
                w_sb = wqkvo[(wname, l)]
                for ko in range(2):
                    ps = psum.tile([128, 512], F32, name="ps_proj", tag="ps_big")
                    for ki in range(2):
                        nc.tensor.matmul(ps[:, :LH], w_sb[:, ki, ko * 128:(ko + 1) * 128],
                                         src[:, ki, :], start=(ki == 0), stop=(ki == 1))
                    nc.scalar.copy(dst[:, ko, :], ps[:, :LH])
            # v row-major, padded per head with a ones column: [128, kt, h, 65]
            v_sb = work.tile([128, 4, NH, 65], BF16, name="v_sb", tag="v_sb")
            nc.gpsimd.memset(v_sb[:, :, :, 64:65], 1.0)
            w_sb = wqkvo[("wv", l)]
            for et, (e0, esz) in enumerate(EVT):
                ps = psum.tile([128, 512], F32, name="ps_v", tag="ps_big")
                for ki in range(2):
                    nc.tensor.matmul(ps[:esz, :256], xT[:, ki, e0:e0 + esz],
                                     w_sb[:, ki, :], start=(ki == 0), stop=(ki == 1))
                nc.scalar.copy(v_sb[:esz, et, :, 0:64],
                               ps[:esz, :256].rearrange("p (h d) -> p h d", h=NH))

            # ---- scores E^T = exp(S^T/8) with causal zeroing ----
            E = work.tile([128, NH, 4, LH], BF16, name="E", tag="E", bufs=1)
            for h in range(NH):
                hp = (64 * h) % 128
                ht = h // 2
                for kt, (k0, ksz) in enumerate(EVT):
                    qlo = k0
                    qn = LH - qlo
                    ps = psum.tile([128, 512], F32, name="ps_s", tag="ps_big")
                    nc.tensor.matmul(ps[:ksz, :qn], kT[hp:hp + 64, ht, k0:k0 + ksz],
                                     qT[hp:hp + 64, ht, qlo:LH], start=True, stop=True)
                    nc.scalar.activation(E[:ksz, h, kt, qlo:LH], ps[:ksz, :qn],
                                         AF.Exp, scale=0.125)
                    # zero strictly-upper part of the diagonal block (q < k)
                    w = min(qn, 128)
                    nc.gpsimd.affine_select(
                        out=E[:ksz, h, kt, qlo:qlo + w], in_=E[:ksz, h, kt, qlo:qlo + w],
                        compare_op=ALU.is_ge, fill=0.0, base=0,
                        pattern=[[1, w]], channel_multiplier=-1)

            # ---- O = softmax @ V (row-major) ----
            attn = work.tile([128, 4, 256], BF16, name="attn", tag="attn")
            for qt, (q0, qsz) in enumerate(EVT):
                ps_o = psum.tile([128, NH, 65], F32, name="ps_o", tag="ps_o", bufs=1)
                for h in range(NH):
                    for kt in range(qt + 1):
                        k0, ksz = EVT[kt]
                        nc.tensor.matmul(ps_o[:qsz, h, :], E[:ksz, h, kt, q0:q0 + qsz],
                                         v_sb[:ksz, kt, h, :],
                                         start=(kt == 0), stop=(kt == qt))
                rs = work.tile([128, NH, 1], F32, name="rs", tag="rs")
                nc.vector.reciprocal(rs[:qsz], ps_o[:qsz, :, 64:65])
                for h in range(NH):
                    nc.vector.tensor_scalar_mul(attn[:qsz, qt, 64 * h:64 * (h + 1)],
                                                ps_o[:qsz, h, 0:64], rs[:qsz, h])

            # ---- attn^T via PE transpose ----
            attnT = work.tile([128, 2, LH], BF16, name="attnT", tag="attnT")
            for ft in range(2):
                for qt, (q0, qsz) in enumerate(EVT):
                    ps_t = psum.tile([128, 128], BF16, name="ps_t", tag="ps_t", bufs=1)
                    nc.tensor.transpose(ps_t[:128, :qsz],
                                        attn[:qsz, qt, ft * 128:(ft + 1) * 128],
                                        ident16[:qsz, :qsz])
                    nc.scalar.copy(attnT[:, ft, q0:q0 + qsz], ps_t[:128, :qsz])

            # ---- cur = cur + attn @ Wo (feature-major) ----
            w_sb = wqkvo[("wo", l)]
            ncurT = acts.tile([128, 2, LH], BF16, name=f"c{l + 1}T{b}")
            for ko in range(2):
                ps = psum.tile([128, 512], F32, name="ps_wo", tag="ps_big")
                for ki in range(2):
                    nc.tensor.matmul(ps[:, :LH], w_sb[:, ki, ko * 128:(ko + 1) * 128],
                                     attnT[:, ki, :], start=(ki == 0), stop=(ki == 1))
                nc.vector.tensor_add(ncurT[:, ko, :], ps[:, :LH], curT[:, ko, :])
            curT = ncurT
            layer_T.append(ncurT)

        c1T, c2T = layer_T

        # ---- enc row-major (for the scatter matmul) ----
        enc_row = acts.tile([128, 4, D_TOTAL], BF16, name=f"enc_row{b}")
        for l, cT in enumerate(layer_T):
            for ft in range(2):
                for qt, (q0, qsz) in enumerate(EVT):
                    ps_t = psum.tile([128, 128], BF16, name="ps_t2", tag="ps_t", bufs=1)
                    nc.tensor.transpose(ps_t[:qsz, :128], cT[:, ft, q0:q0 + qsz],
                                        ident16[:, :128])
                    nc.scalar.copy(enc_row[:qsz, qt, l * 256 + ft * 128:l * 256 + (ft + 1) * 128],
                                   ps_t[:qsz, :128])

        # ---- scatter-mean -> rep^T [1024, 64] (enc block on device, s/r block host) ----
        asc_sb = grp.tile([128, 4, G], BF16, name="asc", tag=f"asc{b}")
        nc.sync.dma_start(asc_sb[:], t["asc"][b].rearrange("(et p) g -> p et g", p=128))
        repT = acts.tile([128, 8, G], BF16, name=f"repT{b}")
        nc.sync.dma_start(repT[:, 4:8, :],
                          t["repsrt"][b].rearrange("(ft p) g -> p ft g", p=128))
        for ft in range(4):
            ps = psum.tile([128, G], F32, name="ps_rep", tag="ps_o", bufs=1)
            for et, (e0, esz) in enumerate(EVT):
                nc.tensor.matmul(ps[:, :], enc_row[:esz, et, ft * 128:(ft + 1) * 128],
                                 asc_sb[:esz, et, :], start=(et == 0), stop=(et == 3))
            nc.scalar.copy(repT[:, ft, :], ps[:, :])

        # ---- gp (both layouts) ----
        gpT = grp.tile([GPD, GPD], BF16, name="gpT", tag=f"gpT{b}")
        ps1 = psum.tile([GPD, GPD], F32, name="ps_gpT", tag="ps_o", bufs=1)
        for kt in range(8):
            nc.tensor.matmul(ps1[:, :], gpw_sb[:, kt, :], repT[:, kt, :],
                             start=(kt == 0), stop=(kt == 7))
        nc.scalar.activation(gpT[:], ps1[:], AF.Identity, bias=gpbc_sb[:])
        gp_r = grp.tile([GPD, GPD], F32, name="gp_r", tag=f"gp_r{b}")
        ps2 = psum.tile([GPD, GPD], F32, name="ps_gp", tag="ps_o", bufs=1)
        for kt in range(8):
            nc.tensor.matmul(ps2[:, :], repT[:, kt, :], gpw_sb[:, kt, :],
                             start=(kt == 0), stop=False)
        nc.tensor.matmul(ps2[:, :], ones_row[:, :GPD], gpbr_sb[:, :],
                         start=False, stop=True)
        nc.vector.tensor_copy(gp_r[:], ps2[:])

        # ---- group attention ----
        qkv = grp.tile([GPD, 3 * GPD], BF16, name="qkv", tag=f"qkv{b}")
        ps = psum.tile([GPD, 3 * GPD], F32, name="ps_qkv", tag="ps_big")
        nc.tensor.matmul(ps[:, :], gpT[:], small_w["gainw"][:], start=True, stop=False)
        nc.tensor.matmul(ps[:, :], ones_row[:, :GPD], small_w["gainbr"][:],
                         start=False, stop=True)
        nc.scalar.copy(qkv[:], ps[:, :])
        # per-head transposed q/k: j=0,1 -> q heads, j=2,3 -> k heads
        qkT = grp.tile([32, 4, GPD], BF16, name="qkT", tag=f"qkT{b}")
        for j in range(4):
            ps_t = psum.tile([32, 128], BF16, name="ps_qt", tag="ps_t", bufs=1)
            nc.tensor.transpose(ps_t[:32, :GPD], qkv[:, 32 * j:32 * (j + 1)],
                                ident16[:GPD, :GPD])
            nc.scalar.copy(qkT[:, j, :], ps_t[:32, :GPD])

        gaT = grp.tile([GPD, GPD], BF16, name="gaT", tag=f"gaT{b}")
        for h in range(GH):
            ps_s = psum.tile([GPD, GPD], F32, name="ps_sg", tag="ps_o", bufs=1)
            nc.tensor.matmul(ps_s[:, :], qkT[:, h, :], qkT[:, 2 + h, :],
                             start=True, stop=True)
            P_sb = grp.tile([GPD, GPD], F32, name="P_sb", tag="P_sb")
            sum_c = grp.tile([GPD, 1], F32, name="sum_c", tag="sum_c")
            nc.scalar.activation(P_sb[:], ps_s[:, :], AF.Exp,
                                 scale=float(1.0 / np.sqrt(32.0)),
                                 accum_out=sum_c[:])
            rsum = grp.tile([GPD, 1], F32, name="rsum", tag="rsum")
            nc.vector.reciprocal(rsum[:], sum_c[:])
            Pn = grp.tile([GPD, GPD], BF16, name="Pn", tag="Pn")
            nc.vector.tensor_scalar_mul(Pn[:], P_sb[:], rsum[:])
            ps_pt = psum.tile([GPD, GPD], BF16, name="ps_pt", tag="ps_t", bufs=1)
            nc.tensor.transpose(ps_pt[:, :], Pn[:], ident16[:GPD, :GPD])
            PnT = grp.tile([GPD, GPD], BF16, name="PnT", tag="PnT")
            nc.scalar.copy(PnT[:], ps_pt[:, :])
            ps_og = psum.tile([32, GPD], F32, name="ps_og", tag="ps_t", bufs=1)
            nc.tensor.matmul(ps_og[:, :], qkv[:, 128 + 32 * h:128 + 32 * (h + 1)],
                             PnT[:], start=True, stop=True)
            nc.scalar.copy(gaT[32 * h:32 * (h + 1), :], ps_og[:, :])

        ps_ga = psum.tile([GPD, GPD], F32, name="ps_ga", tag="ps_o", bufs=1)
        nc.tensor.matmul(ps_ga[:, :], gaT[:], small_w["gaoutw"][:], start=True, stop=False)
        nc.tensor.matmul(ps_ga[:, :], ones_row[:, :GPD], small_w["gaoutbr"][:],
                         start=False, stop=True)
        y1 = grp.tile([GPD, GPD], F32, name="y1", tag=f"y1{b}")
        nc.vector.tensor_add(y1[:], ps_ga[:, :], gp_r[:])

        def layer_norm(y, wt, bt, out_dtype, out_name):
            mu = grp.tile([GPD, 1], F32, name=f"{out_name}_mu", tag=f"{out_name}_mu")
            nc.vector.reduce_sum(mu[:], y[:], axis=AX.X)
            nc.scalar.mul(mu[:], mu[:], 1.0 / GPD)
            xc = grp.tile([GPD, GPD], F32, name=f"{out_name}_xc", tag=f"{out_name}_xc")
            nc.vector.tensor_scalar_sub(xc[:], y[:], mu[:])
            sq = grp.tile([GPD, GPD], F32, name=f"{out_name}_sq", tag=f"{out_name}_sq")
            vs = grp.tile([GPD, 1], F32, name=f"{out_name}_vs", tag=f"{out_name}_vs")
            nc.scalar.activation(sq[:], xc[:], AF.Square, accum_out=vs[:])
            # rstd = exp(-0.5 * ln(var + eps)); avoids the Sqrt act table
            lv = grp.tile([GPD, 1], F32, name=f"{out_name}_lv", tag=f"{out_name}_lv")
            nc.scalar.activation(lv[:], vs[:], AF.Ln, scale=1.0 / GPD, bias=eps_c[:])
            rstd = grp.tile([GPD, 1], F32, name=f"{out_name}_rstd", tag=f"{out_name}_rstd")
            nc.scalar.activation(rstd[:], lv[:], AF.Exp, scale=-0.5)
            xh = grp.tile([GPD, GPD], F32, name=f"{out_name}_xh", tag=f"{out_name}_xh")
            nc.vector.scalar_tensor_tensor(xh[:], xc[:], rstd[:], wt[:],
                                           op0=ALU.mult, op1=ALU.mult)
            out = grp.tile([GPD, GPD], out_dtype, name=out_name, tag=f"{out_name}{b}")
            nc.vector.tensor_add(out[:], xh[:], bt[:])
            return out

        gn = layer_norm(y1, small_w["n1wt"], small_w["n1bt"], F32, "gn")
        ps_gt = psum.tile([GPD, GPD], F32, name="ps_gnT", tag="ps_t", bufs=1)
        nc.tensor.transpose(ps_gt[:, :], gn[:], ident32[:GPD, :GPD])
        gnT = grp.tile([GPD, GPD], BF16, name="gnT", tag=f"gnT{b}")
        nc.scalar.copy(gnT[:], ps_gt[:, :])

        ps_f1 = psum.tile([GPD, GPD], F32, name="ps_f1", tag="ps_o", bufs=1)
        nc.tensor.matmul(ps_f1[:, :], gnT[:], small_w["ffw1"][:], start=True, stop=False)
        nc.tensor.matmul(ps_f1[:, :], ones_row[:, :GPD], small_w["ffb1r"][:],
                         start=False, stop=True)
        ff1 = grp.tile([GPD, GPD], BF16, name="ff1", tag="ff1")
        nc.scalar.activation(ff1[:], ps_f1[:, :], AF.Relu)
        ps_ft = psum.tile([GPD, GPD], BF16, name="ps_ff1T", tag="ps_t", bufs=1)
        nc.tensor.transpose(ps_ft[:, :], ff1[:], ident16[:GPD, :GPD])
        ff1T = grp.tile([GPD, GPD], BF16, name="ff1T", tag="ff1T")
        nc.scalar.copy(ff1T[:], ps_ft[:, :])
        ps_f2 = psum.tile([GPD, GPD], F32, name="ps_f2", tag="ps_o", bufs=1)
        nc.tensor.matmul(ps_f2[:, :], ff1T[:], small_w["ffw2"][:], start=True, stop=False)
        nc.tensor.matmul(ps_f2[:, :], ones_row[:, :GPD], small_w["ffb2r"][:],
                         start=False, stop=True)
        y2 = grp.tile([GPD, GPD], F32, name="y2", tag=f"y2{b}")
        nc.vector.tensor_add(y2[:], ps_f2[:, :], gn[:])

        gout = layer_norm(y2, small_w["n2wt"], small_w["n2bt"], BF16, "gout")

        # ---- gather back to events: gathered^T [64, 511] ----
        a01t_sb = grp.tile([G, LH], BF16, name="a01t", tag=f"a01t{b}")
        nc.sync.dma_start(a01t_sb[:], t["a01t"][b])
        gatheredT = acts.tile([G, LH], BF16, name=f"gatheredT{b}")
        ps_gth = psum.tile([GPD, 512], F32, name="ps_gth", tag="ps_big")
        nc.tensor.matmul(ps_gth[:, :LH], gout[:], a01t_sb[:], start=True, stop=True)
        nc.scalar.copy(gatheredT[:], ps_gth[:, :LH])

        # ---- s/r embedding broadcast rows of merged^T ----
        srt_sb = grp.tile([128, 4, 1], F32, name="srt", tag=f"srt{b}")
        nc.sync.dma_start(srt_sb[:], t["srt"][b].rearrange("(j p) o -> p j o", p=128))
        srb = acts.tile([128, 4, LH], BF16, name=f"srb{b}")
        for j in range(4):
            nc.vector.tensor_scalar_mul(srb[:, j, :], ones_ev[:, :LH], srt_sb[:, j])

        merged_src[b] = [
            (c1T[:, 0, :], 128), (c1T[:, 1, :], 128),
            (c2T[:, 0, :], 128), (c2T[:, 1, :], 128),
            (srb[:, 0, :], 128), (srb[:, 1, :], 128),
            (srb[:, 2, :], 128), (srb[:, 3, :], 128),
            (gatheredT[:, :], 64),
        ]

        # ---- merge matmul -> enhanced^T [1024, 511] ----
        eT = acts.tile([128, 8, LH], BF16, name=f"enhT{b}")
        for fo in range(8):
            ps = psum.tile([128, 512], F32, name="ps_mg", tag="ps_big")
            for kt in range(9):
                src, ksz = merged_src[b][kt]
                nc.tensor.matmul(ps[:, :LH], mgw_sb[:ksz, kt, fo * 128:(fo + 1) * 128],
                                 src, start=(kt == 0), stop=(kt == 8))
            nc.scalar.activation(eT[:, fo, :], ps[:, :LH], AF.Identity,
                                 bias=mgbc_sb[:, fo])
        enhT[b] = eT

    # ---- intensity head, streaming int_w ----
    dma_eng = [nc.sync, nc.scalar]  # alternate HWDGE issuers to spread queues
    for c in range(NCH):
        intw_t = stream.tile([128, 8, ECH], BF16, name="intw_t", tag="intw", bufs=2)
        dma_eng[c % 2].dma_start(intw_t[:],
                                 t["intw"][:, c * ECH:(c + 1) * ECH]
                                 .rearrange("(kt p) e -> p kt e", p=128))
        intb_t = stream.tile([1, ECH], BF16, name="intb_t", tag="intb", bufs=2)
        nc.sync.dma_start(intb_t[:], t["intbr"][:, c * ECH:(c + 1) * ECH])
        for b in range(BL):
            for qt, (q0, qsz) in enumerate(EVT):
                ps = psum.tile([128, 2, 512], F32, name="ps_int", tag="ps_int")
                for j in range(2):
                    for kt in range(8):
                        nc.tensor.matmul(ps[:qsz, j, :500],
                                         enhT[b][:, kt, q0:q0 + qsz],
                                         intw_t[:, kt, j * 500:(j + 1) * 500],
                                         start=(kt == 0), stop=False)
                    nc.tensor.matmul(ps[:qsz, j, :500], ones_row[:, :qsz],
                                     intb_t[:, j * 500:(j + 1) * 500],
                                     start=False, stop=True)
                # softplus(x) = ln(exp(x) + 1); logits are O(1) so no overflow
                e_t = outp.tile([128, 2, 500], F32, name="e_t", tag="e_t", bufs=2)
                nc.scalar.activation(e_t[:qsz], ps[:qsz, :, :500], AF.Exp)
                o_t = outp.tile([128, ECH], BF16, name="o_t", tag="o_t")
                nc.scalar.activation(o_t[:qsz].rearrange("p (j e) -> p j e", j=2),
                                     e_t[:qsz], AF.Ln, bias=1.0)
                dma_eng[(c + b) % 2].dma_start(
                    t["out"][b, q0:q0 + qsz, c * ECH:(c + 1) * ECH], o_t[:qsz])
    ctx.close()


def _patch_single_act_table():
    """Every activation func this kernel uses (Exp, Ln, Copy, Identity,
    Square, Relu) lives in the natural_log_exp_and_others table. The default
    per-instruction chooser bounces between tables, costing an ACT_TABLE_LOAD
    (~1.3us) per switch. Blank out every other table (indices preserved) so a
    single load serves the entire program."""
    import concourse.hw_specs as hw_specs
    orig = hw_specs.get_activation_tables

    def single(arch):
        tabs = orig(arch)
        return {name: (s if name == "natural_log_exp_and_others" else set())
                for name, s in tabs.items()}

    bacc.get_activation_tables = single


def _build_program():
    _patch_single_act_table()
    nc = bacc.Bacc("TRN2", target_bir_lowering=False, debug=False)
    d = {}

    def inp(name, shape, dt):
        d[name] = nc.dram_tensor(name, list(shape), dt, kind="ExternalInput")

    inp("xt", (BL, D, LH), BF16)
    inp("c0t", (BL, D, LH), BF16)
    for w in ("wq", "wk", "wv", "wo"):
        inp(w, (NL, D, D), BF16)
    inp("gpw", (D_FEAT, GPD), BF16)
    inp("gpbr", (1, GPD), BF16)
    inp("gpbc", (GPD, 1), F32)
    inp("gainw", (GPD, 3 * GPD), BF16)
    inp("gainbr", (1, 3 * GPD), BF16)
    inp("gaoutw", (GPD, GPD), BF16)
    inp("gaoutbr", (1, GPD), BF16)
    inp("ffw1", (GPD, GPD), BF16)
    inp("ffb1r", (1, GPD), BF16)
    inp("ffw2", (GPD, GPD), BF16)
    inp("ffb2r", (1, GPD), BF16)
    for n in ("n1wt", "n1bt", "n2wt", "n2bt"):
        inp(n, (GPD, GPD), F32)
    inp("mgw", (D_MG, D_FEAT), BF16)
    inp("mgbc", (D_FEAT, 1), F32)
    inp("intw", (D_FEAT, N_ENTITY), BF16)
    inp("intbr", (1, N_ENTITY), BF16)
    inp("asc", (BL, 512, G), BF16)
    inp("a01t", (BL, G, LH), BF16)
    inp("srt", (BL, 512, 1), F32)
    inp("repsrt", (BL, 512, G), BF16)
    d["out"] = nc.dram_tensor("out", [BL, LH, N_ENTITY], BF16, kind="ExternalOutput")

    with tile.TileContext(nc) as tc:
        _emit(nc, tc, d)
    nc.compile()
    return nc


_PROG = None


def _get_program():
    global _PROG
    if _PROG is None:
        _PROG = _build_program()
    return _PROG


# -------------------------------------------------------------------- kernel

def prepare_in_maps(subs, marks, objs, times, dt, mask, group_map,
                    obj_embed, core_Wq, core_Wk, core_Wv, core_Wo,
                    sub_embed, rel_embed, gp_w, gp_b, ga_in_w, ga_in_b,
                    ga_out_w, ga_out_b, ffn_w1, ffn_b1, ffn_w2, ffn_b2,
                    n1_w, n1_b, n2_w, n2_b, mg_w, mg_b, int_w, int_b):
    subs = np.asarray(subs).astype(np.int64)
    marks = np.asarray(marks).astype(np.int64)
    objs = np.asarray(objs).astype(np.int64)
    times = _f32(times)
    dt = _f32(dt)
    mask = np.asarray(mask).astype(bool)
    group_map = np.asarray(group_map).astype(np.int64)
    obj_embed = _f32(obj_embed)
    sub_embed = _f32(sub_embed)
    rel_embed = _f32(rel_embed)

    hist = objs[:, :-1]
    x = obj_embed[hist] + _time_enc(times[:, :-1]) + _time_enc(dt[:, :-1])
    cur0 = _time_enc(times[:, 1:])
    xT = _bf16(x.transpose(0, 2, 1))          # [B, 256, 511]
    c0T = _bf16(cur0.transpose(0, 2, 1))

    g_ids = group_map[subs[:, :-1] * N_REL + marks[:, :-1]]   # [B, 511]
    fm = mask[:, :-1].astype(np.float32)
    a01 = np.zeros((B, LH, G), np.float32)
    bi = np.arange(B)[:, None].repeat(LH, 1)
    ei = np.arange(LH)[None, :].repeat(B, 0)
    a01[bi.ravel(), ei.ravel(), g_ids.ravel()] = fm.ravel()
    cnt = a01.sum(axis=1)                                     # [B, G]
    asc = a01 / np.maximum(cnt, 1.0)[:, None, :]
    asc_pad = np.zeros((B, 512, G), np.float32)
    asc_pad[:, :LH] = asc
    a01t = a01.transpose(0, 2, 1)                             # [B, G, 511]

    s_emb = sub_embed[subs[:, 0]]                             # [B, 256]
    r_emb = rel_embed[marks[:, 0]]                            # [B, 256]
    sr = np.concatenate([s_emb, r_emb], axis=1)               # [B, 512]
    ind = (cnt > 0).astype(np.float32)                        # [B, G]
    repsrt = sr[:, :, None] * ind[:, None, :]                 # [B, 512, G]

    shared = {
        "wq": _bf16(core_Wq), "wk": _bf16(core_Wk),
        "wv": _bf16(core_Wv), "wo": _bf16(core_Wo),
        "gpw": _bf16(gp_w), "gpbr": _bf16(np.asarray(gp_b).reshape(1, GPD)),
        "gpbc": _f32(np.asarray(gp_b).reshape(GPD, 1)),
        "gainw": _bf16(ga_in_w), "gainbr": _bf16(np.asarray(ga_in_b).reshape(1, -1)),
        "gaoutw": _bf16(ga_out_w), "gaoutbr": _bf16(np.asarray(ga_out_b).reshape(1, -1)),
        "ffw1": _bf16(ffn_w1), "ffb1r": _bf16(np.asarray(ffn_b1).reshape(1, -1)),
        "ffw2": _bf16(ffn_w2), "ffb2r": _bf16(np.asarray(ffn_b2).reshape(1, -1)),
        "n1wt": _f32(np.broadcast_to(np.asarray(n1_w, np.float32), (GPD, GPD))),
        "n1bt": _f32(np.broadcast_to(np.asarray(n1_b, np.float32), (GPD, GPD))),
        "n2wt": _f32(np.broadcast_to(np.asarray(n2_w, np.float32), (GPD, GPD))),
        "n2bt": _f32(np.broadcast_to(np.asarray(n2_b, np.float32), (GPD, GPD))),
        "mgw": _bf16(mg_w), "mgbc": _f32(np.asarray(mg_b).reshape(D_FEAT, 1)),
        "intw": _bf16(int_w), "intbr": _bf16(np.asarray(int_b).reshape(1, N_ENTITY)),
    }

    in_maps = []
    for core in range(NCORES):
        bs = slice(core * BL, (core + 1) * BL)
        m = dict(shared)
        m["xt"] = xT[bs]
        m["c0t"] = c0T[bs]
        m["asc"] = _bf16(asc_pad[bs])
        m["a01t"] = _bf16(a01t[bs])
        m["srt"] = _f32(sr[bs][:, :, None])
        m["repsrt"] = _bf16(repsrt[bs])
        in_maps.append(m)
    return in_maps


def kernel(**inputs):
    global LAST_RESULTS, LAST_RUN_WALL_NS
    import time
    in_maps = prepare_in_maps(**inputs)
    nc = _get_program()
    t0 = time.perf_counter()
    res = run_bass_kernel_spmd(nc, in_maps, core_ids=list(range(NCORES)))
    LAST_RUN_WALL_NS = int((time.perf_counter() - t0) * 1e9)
    LAST_RESULTS = res
    out = np.concatenate([res.results[i]["out"] for i in range(NCORES)], axis=0)
    return np.ascontiguousarray(out.reshape(B, LH, N_ENTITY).astype(np.float32))
